# revision 1
# baseline (speedup 1.0000x reference)
"""VQ-EMA codebook update kernel for Trainium2, 8 NeuronCores.

Strategy (data-parallel over tokens, per the standard VQ-EMA sync):
  - Each core gets N/8 = 4096 tokens; the [K=8192, C=384] dictionary is replicated.
  - P0: normalize dictionary rows (fp32, Newton-refined rsqrt), transpose via PE
        into ndT [C, K] stored as float32r (rounded) for fast PE matmuls.
  - P1: per 128-token tile: transpose RAW tokens (argmax over k is invariant to
        positive per-row scaling, so features need no normalization),
        sim = xT.T @ ndT in fp32r, PSUM chunks drained to a bf16 sim row,
        rowmax via tensor-tensor max tree (2x DVE mode) + one 4x-mode is_ge
        for the multi-hot row, spilled to DRAM (bf16).
  - P2: segment sums: for each K-tile, accumulate onehot.T @ [x|1] (bf16) over all
        token tiles in PSUM -> partial [K, C+1] (feature sums + counts).
  - P3: ReduceScatter(add) across the 8 cores -> each core owns a [1024, C+1]
        shard; EMA update + where(used) blend on-core; output its dict shard.
Host: shards inputs, concatenates the 8 output shards.
"""

import sys

sys.path.insert(0, "/opt/trn_rl_repo")

import functools

import numpy as np

N = 32768
C = 384
K = 8192
NCORES = 8
NSH = N // NCORES  # 4096 tokens per core
KSH = K // NCORES  # 1024 dict rows per core
TT = NSH // 128  # 32 token tiles per core
KT = K // 128  # 64 K tiles
CB = C // 128  # 3 contraction chunks
SIMW = 512  # sim matmul free width (PSUM bank, fp32)
NSIMW = K // SIMW  # 16 chunks
XW = C + 1  # 385: x plus ones column
MOM = 0.99


@functools.cache
def _build(reps: int = 1):
    import concourse.bacc as bacc
    import concourse.masks as masks
    import concourse.mybir as mybir
    import concourse.tile as tile

    f32 = mybir.dt.float32
    f32r = mybir.dt.float32r
    bf16 = mybir.dt.bfloat16

    nc = bacc.Bacc("TRN2", target_bir_lowering=False, debug=False, num_devices=NCORES)

    feat = nc.dram_tensor("feat", [NSH, C], f32, kind="ExternalInput").ap()
    dic = nc.dram_tensor("dic", [K, C], f32, kind="ExternalInput").ap()
    dsum = nc.dram_tensor("dsum", [KSH, C], f32, kind="ExternalInput").ap()
    dnum = nc.dram_tensor("dnum", [KSH // 128, 128, 1], f32, kind="ExternalInput").ap()
    dsh = nc.dram_tensor("dsh", [KSH, C], f32, kind="ExternalInput").ap()
    out_shard = nc.dram_tensor("out_shard", [KSH, C], f32, kind="ExternalOutput").ap()

    with tile.TileContext(nc) as tc:
        for rep in range(reps):
            with (
                tc.tile_pool(name="constp", bufs=1) as constp,
                tc.tile_pool(name="mainp", bufs=1) as mainp,
                tc.tile_pool(name="dramp", bufs=1, space="DRAM") as dramp,
            ):
                ident = constp.tile([128, 128], f32, name="ident")
                masks.make_identity(nc, ident[:])

                # Persistent SBUF tensors
                ndT = [
                    mainp.tile([128, K], f32r, name=f"ndT{c}_r{rep}", uniquify=False)
                    for c in range(CB)
                ]
                xe_sb = [
                    mainp.tile([128, XW], bf16, name=f"xe{t}_r{rep}", uniquify=False)
                    for t in range(TT)
                ]
                # DRAM scratch
                onehot_dram = dramp.tile([NSH, K], bf16, name=f"onehot_dram_r{rep}")
                partial_dram = dramp.tile([K, XW], f32, name=f"partial_dram_r{rep}")
                ccout_dram = dramp.tile([KSH, XW], f32, name=f"ccout_dram_r{rep}")

                def rsqrt_refined(pool, ss, tag):
                    """r ~= 1/sqrt(ss), fp32-accurate via 2 Newton steps on [128,1]."""
                    rec = pool.tile([128, 1], f32, name=f"rec_{tag}", tag="rec")
                    r = pool.tile([128, 1], f32, name=f"r_{tag}", tag="r")
                    t = pool.tile([128, 1], f32, name=f"t_{tag}", tag="t")
                    nc.vector.reciprocal(rec[:], ss[:])
                    nc.scalar.sqrt(r[:], rec[:])
                    for _ in range(2):
                        nc.vector.tensor_tensor(t[:], r[:], r[:], mybir.AluOpType.mult)
                        nc.vector.tensor_tensor(t[:], t[:], ss[:], mybir.AluOpType.mult)
                        # t = 1.5 - 0.5*t
                        nc.vector.tensor_scalar(
                            t[:], t[:], -0.5, 1.5, mybir.AluOpType.mult, mybir.AluOpType.add
                        )
                        nc.vector.tensor_tensor(r[:], r[:], t[:], mybir.AluOpType.mult)
                    return r

                # ---------------- P0: dictionary normalize + transpose ----------------
                with (
                    tc.tile_pool(name="p0sb", bufs=3) as p0sb,
                    tc.tile_pool(name="p0sc", bufs=2) as p0sc,
                    tc.tile_pool(name="p0ps", bufs=2, space="PSUM") as p0ps,
                ):
                    for dt_i in range(KT):
                        d = p0sb.tile([128, C], f32, name="d", tag="d")
                        nc.sync.dma_start(d[:], dic[dt_i * 128 : (dt_i + 1) * 128, :])
                        sq = p0sc.tile([128, C], f32, name="sq", tag="sq")
                        ss = p0sc.tile([128, 1], f32, name="ss", tag="ss")
                        nc.scalar.activation(
                            sq[:], d[:], mybir.ActivationFunctionType.Square, accum_out=ss[:]
                        )
                        r = rsqrt_refined(p0sc, ss, "p0")
                        nd = p0sb.tile([128, C], f32, name="nd", tag="nd")
                        nc.scalar.activation(
                            nd[:], d[:], mybir.ActivationFunctionType.Copy, scale=r[:, 0:1]
                        )
                        for c in range(CB):
                            pst = p0ps.tile([128, 128], f32, name="pst", tag="pst")
                            nc.tensor.transpose(pst[:], nd[:, c * 128 : (c + 1) * 128], ident[:])
                            nc.vector.tensor_copy(
                                ndT[c][:, dt_i * 128 : (dt_i + 1) * 128], pst[:]
                            )

                # ---------------- P1: sim + multi-hot per token tile ----------------
                with (
                    tc.tile_pool(name="p1sb", bufs=3) as p1sb,
                    tc.tile_pool(name="p1rm", bufs=2) as p1rm,
                    tc.tile_pool(name="p1sim", bufs=2) as p1sim,
                    tc.tile_pool(name="p1oh", bufs=1) as p1oh,
                    tc.tile_pool(name="p1ps", bufs=3, space="PSUM") as p1ps,
                    tc.tile_pool(name="p1pst", bufs=2, space="PSUM") as p1pst,
                ):
                    for tt in range(TT):
                        x = p1sb.tile([128, C], f32, name="x", tag="x")
                        nc.sync.dma_start(x[:], feat[tt * 128 : (tt + 1) * 128, :])
                        # raw x (bf16) + ones column, kept resident in SBUF for P2
                        nc.vector.memset(xe_sb[tt][:, C:XW], 1.0)
                        nc.vector.tensor_copy(xe_sb[tt][:, 0:C], x[:])

                        xT = []
                        for c in range(CB):
                            pst = p1pst.tile([128, 128], f32, name="pstx", tag="pstx")
                            nc.tensor.transpose(pst[:], x[:, c * 128 : (c + 1) * 128], ident[:])
                            xc = p1sb.tile([128, 128], f32r, name="xc", tag=f"xc{c}")
                            nc.scalar.copy(xc[:], pst[:])
                            xT.append(xc)

                        simbuf = p1sim.tile([128, K], bf16, name="simbuf", tag="simbuf")
                        for kc in range(NSIMW // 2):
                            # 2-bank PSUM tile; each matmul fills a 512 half,
                            # one wide Act drain per 1024 chunk
                            ps = p1ps.tile([128, 2 * SIMW], f32, name="ps_sim", tag="ps_sim")
                            for h in range(2):
                                kw = kc * 2 + h
                                for c in range(CB):
                                    nc.tensor.matmul(
                                        ps[:, h * SIMW : (h + 1) * SIMW],
                                        xT[c][:],
                                        ndT[c][:, kw * SIMW : (kw + 1) * SIMW],
                                        start=(c == 0),
                                        stop=(c == CB - 1),
                                    )
                            nc.scalar.copy(
                                simbuf[:, kc * 2 * SIMW : (kc + 1) * 2 * SIMW], ps[:]
                            )

                        # rowmax via TT-max tree (2x DVE mode on bf16) + small reduce
                        t4 = p1rm.tile([128, 4096], bf16, name="t4", tag="t4")
                        nc.vector.tensor_tensor(
                            t4[:], simbuf[:, 0:4096], simbuf[:, 4096:8192],
                            mybir.AluOpType.max,
                        )
                        t2 = p1rm.tile([128, 2048], bf16, name="t2", tag="t2")
                        nc.vector.tensor_tensor(
                            t2[:], t4[:, 0:2048], t4[:, 2048:4096], mybir.AluOpType.max
                        )
                        t1 = p1rm.tile([128, 1024], bf16, name="t1", tag="t1")
                        nc.vector.tensor_tensor(
                            t1[:], t2[:, 0:1024], t2[:, 1024:2048], mybir.AluOpType.max
                        )
                        rowmax = p1rm.tile([128, 1], f32, name="rowmax", tag="rowmax")
                        nc.vector.tensor_reduce(
                            rowmax[:], t1[:], mybir.AxisListType.X, mybir.AluOpType.max
                        )
                        onehot = p1oh.tile([128, K], bf16, name="onehot", tag="onehot")
                        nc.vector.tensor_scalar(
                            onehot[:], simbuf[:], rowmax[:, 0:1], None, mybir.AluOpType.is_ge
                        )
                        nc.sync.dma_start(
                            onehot_dram[tt * 128 : (tt + 1) * 128, :], onehot[:]
                        )

                # ---------------- P2: segment sums via one-hot matmuls ----------------
                with (
                    tc.tile_pool(name="p2oh", bufs=6) as p2oh,
                    tc.tile_pool(name="p3sb", bufs=2) as p3sb,
                    tc.tile_pool(name="p2st", bufs=2) as p2st,
                    tc.tile_pool(name="p2ps", bufs=8, space="PSUM") as p2ps,
                ):
                    for g in range(8):
                        segs = [
                            p2ps.tile([128, XW], f32, name=f"ps_seg{b}", tag="ps_seg")
                            for b in range(8)
                        ]
                        for tt in range(TT):
                            oh = p2oh.tile([128, 1024], bf16, name="oh", tag="oh")
                            nc.sync.dma_start(
                                oh[:],
                                onehot_dram[
                                    tt * 128 : (tt + 1) * 128, g * 1024 : (g + 1) * 1024
                                ],
                            )
                            for b in range(8):
                                nc.tensor.matmul(
                                    segs[b][:],
                                    oh[:, b * 128 : (b + 1) * 128],
                                    xe_sb[tt][:],
                                    start=(tt == 0),
                                    stop=(tt == TT - 1),
                                )
                        for b in range(8):
                            stg = p2st.tile([128, XW], f32, name="stg", tag="stg")
                            nc.scalar.copy(stg[:], segs[b][:])
                            kt = g * 8 + b
                            nc.sync.dma_start(
                                partial_dram[kt * 128 : (kt + 1) * 128, :], stg[:]
                            )
                        # per-group ReduceScatter: overlaps later groups' matmuls on PE.
                        # rank i receives rows [g*1024 + i*128, +128) -> ccout[g*128:(g+1)*128]
                        nc.gpsimd.collective_compute(
                            "ReduceScatter",
                            mybir.AluOpType.add,
                            replica_groups=[list(range(NCORES))],
                            ins=[partial_dram[g * 1024 : (g + 1) * 1024, :].opt()],
                            outs=[ccout_dram[g * 128 : (g + 1) * 128, :].opt()],
                        )
                        st = g
                        red = p3sb.tile([128, XW], f32, name="red", tag="red")
                        nc.sync.dma_start(red[:], ccout_dram[st * 128 : (st + 1) * 128, :])
                        dsum_t = p3sb.tile([128, C], f32, name="dsum_t", tag="dsum_t")
                        nc.sync.dma_start(dsum_t[:], dsum[st * 128 : (st + 1) * 128, :])
                        dnum_t = p3sb.tile([128, 1], f32, name="dnum_t", tag="dnum_t")
                        nc.sync.dma_start(dnum_t[:], dnum[st, :, :])
                        dsh_t = p3sb.tile([128, C], f32, name="dsh_t", tag="dsh_t")
                        nc.sync.dma_start(dsh_t[:], dsh[st * 128 : (st + 1) * 128, :])

                        cnt = red[:, C : C + 1]
                        maskb = p3sb.tile([128, 1], f32, name="maskb", tag="maskb")
                        nc.vector.tensor_scalar(
                            maskb[:], cnt, 0.0, None, mybir.AluOpType.is_gt
                        )
                        mask001 = p3sb.tile([128, 1], f32, name="mask001", tag="mask001")
                        nc.vector.tensor_scalar(
                            mask001[:], cnt, 0.0, 1.0 - MOM,
                            mybir.AluOpType.is_gt, mybir.AluOpType.mult,
                        )
                        tmp = p3sb.tile([128, C], f32, name="tmp", tag="tmp")
                        nc.vector.tensor_tensor(
                            tmp[:], red[:, 0:C], dsum_t[:], mybir.AluOpType.subtract
                        )
                        nc.vector.tensor_scalar(
                            tmp[:], tmp[:], mask001[:, 0:1], None, mybir.AluOpType.mult
                        )
                        nsum = p3sb.tile([128, C], f32, name="nsum", tag="nsum")
                        nc.vector.tensor_tensor(
                            nsum[:], tmp[:], dsum_t[:], mybir.AluOpType.add
                        )
                        n0 = p3sb.tile([128, 1], f32, name="n0", tag="n0")
                        nc.vector.tensor_tensor(
                            n0[:], cnt, dnum_t[:], mybir.AluOpType.subtract
                        )
                        nc.vector.tensor_tensor(
                            n0[:], n0[:], mask001[:], mybir.AluOpType.mult
                        )
                        nnum = p3sb.tile([128, 1], f32, name="nnum", tag="nnum")
                        nc.vector.tensor_tensor(
                            nnum[:], n0[:], dnum_t[:], mybir.AluOpType.add
                        )
                        rec = p3sb.tile([128, 1], f32, name="recq", tag="recq")
                        nc.vector.reciprocal(rec[:], nnum[:])
                        q = p3sb.tile([128, C], f32, name="q", tag="q")
                        nc.vector.tensor_scalar(
                            q[:], nsum[:], rec[:, 0:1], None, mybir.AluOpType.mult
                        )
                        nc.vector.tensor_tensor(
                            q[:], q[:], dsh_t[:], mybir.AluOpType.subtract
                        )
                        nc.vector.tensor_scalar(
                            q[:], q[:], maskb[:, 0:1], None, mybir.AluOpType.mult
                        )
                        outt = p3sb.tile([128, C], f32, name="outt", tag="outt")
                        nc.vector.tensor_tensor(
                            outt[:], q[:], dsh_t[:], mybir.AluOpType.add
                        )
                        nc.sync.dma_start(
                            out_shard[st * 128 : (st + 1) * 128, :], outt[:]
                        )

    nc.compile()
    return nc


def _shard_rows(i):
    """Global dictionary rows owned by core i: the i-th 128-block of each group."""
    return [(g * KSH + i * 128, g * KSH + i * 128 + 128) for g in range(KSH // 128)]


def shard_inputs(feature, dictionary, dictionary_sum, dictionary_num):
    in_maps = []
    for i in range(NCORES):
        rows = _shard_rows(i)
        dsum_i = np.concatenate([dictionary_sum[a:b] for a, b in rows], axis=0)
        dsh_i = np.concatenate([dictionary[a:b] for a, b in rows], axis=0)
        dnum_i = np.concatenate([dictionary_num[a:b] for a, b in rows], axis=0)
        in_maps.append(
            {
                "feat": np.ascontiguousarray(feature[i * NSH : (i + 1) * NSH]),
                "dic": dictionary,
                "dsum": np.ascontiguousarray(dsum_i),
                "dnum": np.ascontiguousarray(dnum_i).reshape(KSH // 128, 128, 1),
                "dsh": np.ascontiguousarray(dsh_i),
            }
        )
    return in_maps


def unshard_output(results):
    out = np.empty((K, C), np.float32)
    for i in range(NCORES):
        rows = _shard_rows(i)
        for g, (a, b) in enumerate(rows):
            out[a:b] = results[i]["out_shard"][g * 128 : (g + 1) * 128]
    return out


def kernel(feature, dictionary, dictionary_sum, dictionary_num):
    from concourse import bass_utils

    feature = np.ascontiguousarray(feature, dtype=np.float32)
    dictionary = np.ascontiguousarray(dictionary, dtype=np.float32)
    dictionary_sum = np.ascontiguousarray(dictionary_sum, dtype=np.float32)
    dictionary_num = np.ascontiguousarray(dictionary_num, dtype=np.float32)

    nc = _build()
    in_maps = shard_inputs(feature, dictionary, dictionary_sum, dictionary_num)
    res = bass_utils.run_bass_kernel_spmd(nc, in_maps, core_ids=list(range(NCORES)))
    return unshard_output(res.results).astype(np.float32)



# revision 21
# speedup vs baseline: 2.8251x; 2.8251x over previous
"""VQ-EMA codebook update kernel for Trainium2, 8 NeuronCores.

Strategy (data-parallel over tokens, per the standard VQ-EMA sync):
  - Each core gets N/8 = 4096 tokens; the [K=8192, C=384] dictionary is replicated.
  - P0: normalize dictionary rows (fp32, Newton-refined rsqrt), transpose via PE
        into ndT [C, K] stored as float32r (rounded) for fast PE matmuls.
  - P1: per 128-token tile: transpose RAW tokens (argmax over k is invariant to
        positive per-row scaling, so features need no normalization),
        sim = xT.T @ ndT in fp32r, PSUM chunks drained to a bf16 sim row,
        rowmax via tensor-tensor max tree (2x DVE mode) + one 4x-mode is_ge
        for the multi-hot row, spilled to DRAM (bf16).
  - P2: segment sums: for each K-tile, accumulate onehot.T @ [x|1] (bf16) over all
        token tiles in PSUM -> partial [K, C+1] (feature sums + counts).
  - P3: ReduceScatter(add) across the 8 cores -> each core owns a [1024, C+1]
        shard; EMA update + where(used) blend on-core; output its dict shard.
Host: shards inputs, concatenates the 8 output shards.
"""

import sys

sys.path.insert(0, "/opt/trn_rl_repo")

import functools

import numpy as np

N = 32768
C = 384
K = 8192
NCORES = 8
NSH = N // NCORES  # 4096 tokens per core
KSH = K // NCORES  # 1024 dict rows per core
TT = NSH // 128  # 32 token tiles per core
KT = K // 128  # 64 K tiles
CB = C // 128  # 3 contraction chunks
SIMW = 512  # sim matmul free width (PSUM bank, fp32)
NSIMW = K // SIMW  # 16 chunks
XW = C + 1  # 385: x plus ones column
XWP = 400  # XW padded so the DoubleRow pair-step stays 16B-aligned
MOM = 0.99


@functools.cache
def _build(reps: int = 1, sim_single_core: bool = False):
    import concourse.bacc as bacc
    import concourse.masks as masks
    import concourse.mybir as mybir
    import concourse.tile as tile

    f32 = mybir.dt.float32
    f32r = mybir.dt.float32r
    bf16 = mybir.dt.bfloat16
    f8 = mybir.dt.float8e4
    DR = mybir.MatmulPerfMode.DoubleRow

    nc = bacc.Bacc(
        "TRN2",
        target_bir_lowering=False,
        debug=False,
        num_devices=1 if sim_single_core else NCORES,
    )

    feat = nc.dram_tensor("feat", [NSH, C], f32, kind="ExternalInput").ap()
    dic = nc.dram_tensor("dic", [K, C], f32, kind="ExternalInput").ap()
    dsum = nc.dram_tensor("dsum", [KSH, C], f32, kind="ExternalInput").ap()
    dnum = nc.dram_tensor("dnum", [KSH // 128, 128, 1], f32, kind="ExternalInput").ap()
    dsh = nc.dram_tensor("dsh", [KSH, C], f32, kind="ExternalInput").ap()
    out_shard = nc.dram_tensor("out_shard", [KSH, C], f32, kind="ExternalOutput").ap()

    with tile.TileContext(nc) as tc:
        for rep in range(reps):
            with (
                tc.tile_pool(name="constp", bufs=1) as constp,
                tc.tile_pool(name="mainp", bufs=1) as mainp,
                tc.tile_pool(name="dramp", bufs=1, space="DRAM") as dramp,
            ):
                ident = constp.tile([128, 128], f32, name="ident")
                masks.make_identity(nc, ident[:])

                # Persistent SBUF tensors
                ndT = [
                    mainp.tile([128, K], bf16, name=f"ndT{c}_r{rep}", uniquify=False)
                    for c in range(CB)
                ]
                # fp8 x||1 in DoubleRow pair layout: [:, i, :] = token tile 2p+i
                xe_sb = [
                    mainp.tile([128, 2, XWP], f8, name=f"xe{t}_r{rep}", uniquify=False)
                    for t in range(TT // 2)
                ]
                # onehot spill, one tile per token-tile PAIR: [:, i, :] = tile 2p+i
                # (split tiles keep P2 loads from depending on later P1 spills)
                onehot_dram = [
                    dramp.tile([128, 2, K], f8, name=f"ohd{p}_r{rep}")
                    for p in range(TT // 2)
                ]
                partial_dram = dramp.tile([K, XW], f32, name=f"partial_dram_r{rep}")
                ccout_dram = dramp.tile([KSH, XW], f32, name=f"ccout_dram_r{rep}")

                def rsqrt_refined(pool, ss, tag, w=1):
                    """r ~= 1/sqrt(ss), fp32-accurate via 2 Newton steps on [128,w]."""
                    rec = pool.tile([128, w], f32, name=f"rec_{tag}", tag="rec")
                    r = pool.tile([128, w], f32, name=f"r_{tag}", tag="r")
                    t = pool.tile([128, w], f32, name=f"t_{tag}", tag="t")
                    nc.vector.reciprocal(rec[:], ss[:])
                    nc.scalar.sqrt(r[:], rec[:])
                    for _ in range(2):
                        nc.vector.tensor_tensor(t[:], r[:], r[:], mybir.AluOpType.mult)
                        nc.vector.tensor_tensor(t[:], t[:], ss[:], mybir.AluOpType.mult)
                        # t = 1.5 - 0.5*t
                        nc.vector.tensor_scalar(
                            t[:], t[:], -0.5, 1.5, mybir.AluOpType.mult, mybir.AluOpType.add
                        )
                        nc.vector.tensor_tensor(r[:], r[:], t[:], mybir.AluOpType.mult)
                    return r

                # ---------------- P0: dictionary normalize + transpose ----------------
                # All 64 dict tiles load (cast f32->bf16), squares reduce into
                # one [128, 64] table, ONE batched Newton rsqrt, then per-tile
                # scale-copy + XBAR DMA-transposes straight into ndT (no PE).
                with (
                    tc.tile_pool(name="p0d", bufs=1) as p0d,
                    tc.tile_pool(name="p0sc", bufs=3) as p0sc,
                ):
                    sstab = p0sc.tile([128, KT], f32, name="sstab", tag="sstab")
                    dbs = []
                    for k in range(KT):
                        db = p0d.tile([128, C], bf16, name=f"db{k}")
                        nc.gpsimd.dma_start(db[:], dic[k * 128 : (k + 1) * 128, :])
                        sqd = p0sc.tile([128, C], bf16, name="sqd", tag="sqd")
                        nc.vector.tensor_tensor_reduce(
                            sqd[:], db[:], db[:], 1.0, 0.0,
                            mybir.AluOpType.mult, mybir.AluOpType.add,
                            accum_out=sstab[:, k : k + 1],
                        )
                        dbs.append(db)
                    r = rsqrt_refined(p0sc, sstab, "p0", KT)
                    for k in range(KT):
                        ndb = p0sc.tile([128, C], bf16, name="ndb", tag="ndb")
                        nc.scalar.activation(
                            ndb[:], dbs[k][:], mybir.ActivationFunctionType.Copy,
                            scale=r[:, k : k + 1],
                        )
                        for c in range(CB):
                            nc.sync.dma_start(
                                ndT[c][:, k * 128 : (k + 1) * 128],
                                ndb[:, c * 128 : (c + 1) * 128],
                                transpose=True,
                            )

                # ---------------- P1: sim + multi-hot per token tile ----------------
                with (
                    tc.tile_pool(name="p1sb", bufs=3) as p1sb,
                    tc.tile_pool(name="p1rm", bufs=2) as p1rm,
                    tc.tile_pool(name="p1sim", bufs=2) as p1sim,
                    tc.tile_pool(name="p1oh", bufs=1) as p1oh,
                    tc.tile_pool(name="p1ps", bufs=3, space="PSUM") as p1ps,
                    tc.tile_pool(name="p1pst", bufs=2, space="PSUM") as p1pst,
                ):
                    def load_xT(tt):
                        """DMA tile tt, stash fp8 x||1, transpose to bf16 xT.

                        Called one iteration ahead so the PE transposes queue
                        before the previous tile's sim matmuls (no PE stall).
                        """
                        x = p1sb.tile([128, C], f32, name="x", tag="x")
                        nc.sync.dma_start(x[:], feat[tt * 128 : (tt + 1) * 128, :])
                        # raw x (fp8) + ones column + zero pad, resident for P2
                        nc.vector.memset(xe_sb[tt // 2][:, tt % 2, C:XWP], 0.0)
                        nc.vector.memset(xe_sb[tt // 2][:, tt % 2, C : C + 1], 1.0)
                        nc.vector.tensor_copy(xe_sb[tt // 2][:, tt % 2, 0:C], x[:])
                        xT = []
                        for c in range(CB):
                            pst = p1pst.tile([128, 128], f32, name="pstx", tag="pstx")
                            nc.tensor.transpose(pst[:], x[:, c * 128 : (c + 1) * 128], ident[:])
                            xc = p1sb.tile([128, 128], bf16, name="xc", tag=f"xc{c}")
                            nc.scalar.copy(xc[:], pst[:])
                            xT.append(xc)
                        return xT

                    xT_next = load_xT(0)
                    for tt in range(TT):
                        xT = xT_next
                        if tt + 1 < TT:
                            xT_next = load_xT(tt + 1)

                        simbuf = p1sim.tile([128, K], bf16, name="simbuf", tag="simbuf")
                        for kc in range(NSIMW // 2):
                            # 2-bank PSUM tile; each matmul fills a 512 half,
                            # one wide Act drain per 1024 chunk
                            ps = p1ps.tile([128, 2 * SIMW], f32, name="ps_sim", tag="ps_sim")
                            for h in range(2):
                                kw = kc * 2 + h
                                for c in range(CB):
                                    nc.tensor.matmul(
                                        ps[:, h * SIMW : (h + 1) * SIMW],
                                        xT[c][:],
                                        ndT[c][:, kw * SIMW : (kw + 1) * SIMW],
                                        start=(c == 0),
                                        stop=(c == CB - 1),
                                    )
                            nc.scalar.copy(
                                simbuf[:, kc * 2 * SIMW : (kc + 1) * 2 * SIMW], ps[:]
                            )

                        # rowmax via TT-max tree (2x DVE mode on bf16) + small reduce
                        t4 = p1rm.tile([128, 4096], bf16, name="t4", tag="t4")
                        nc.vector.tensor_tensor(
                            t4[:], simbuf[:, 0:4096], simbuf[:, 4096:8192],
                            mybir.AluOpType.max,
                        )
                        t2 = p1rm.tile([128, 2048], bf16, name="t2", tag="t2")
                        nc.vector.tensor_tensor(
                            t2[:], t4[:, 0:2048], t4[:, 2048:4096], mybir.AluOpType.max
                        )
                        t1 = p1rm.tile([128, 1024], bf16, name="t1", tag="t1")
                        nc.vector.tensor_tensor(
                            t1[:], t2[:, 0:1024], t2[:, 1024:2048], mybir.AluOpType.max
                        )
                        rowmax = p1rm.tile([128, 1], f32, name="rowmax", tag="rowmax")
                        nc.vector.tensor_reduce(
                            rowmax[:], t1[:], mybir.AxisListType.X, mybir.AluOpType.max
                        )
                        onehot = p1oh.tile([128, K], bf16, name="onehot", tag="onehot")
                        nc.vector.tensor_scalar(
                            onehot[:], simbuf[:], rowmax[:, 0:1], None, mybir.AluOpType.is_ge
                        )
                        # casting spill (bf16 -> fp8): SWDGE (gpsimd) DMAs convert
                        nc.gpsimd.dma_start(
                            onehot_dram[tt // 2][:, tt % 2, :], onehot[:]
                        )

                # ---------------- P2: segment sums via one-hot matmuls ----------------
                with (
                    tc.tile_pool(name="p2oh", bufs=20) as p2oh,
                    tc.tile_pool(name="p3sb", bufs=2) as p3sb,
                    tc.tile_pool(name="p2st", bufs=2) as p2st,
                    tc.tile_pool(name="p2ps", bufs=8, space="PSUM") as p2ps,
                ):
                    NP = TT // 2  # 16 token-tile pairs
                    for g in range(8):
                        # stage this group's onehot slabs once; both half-groups
                        # read from them (cols h*512..h*512+512)
                        ohs = []
                        for p in range(NP):
                            oh = p2oh.tile([128, 2, 1024], f8, name="oh", tag="oh")
                            nc.sync.dma_start(
                                oh[:],
                                onehot_dram[p][:, :, g * 1024 : (g + 1) * 1024],
                            )
                            ohs.append(oh)
                        for h in range(2):
                            segs = [
                                p2ps.tile(
                                    [128, XWP], f32, name=f"ps_seg{h}_{b}", tag="ps_seg"
                                )
                                for b in range(4)
                            ]
                            for p in range(NP):
                                for b in range(4):
                                    nc.tensor.matmul(
                                        segs[b][:],
                                        ohs[p][
                                            :, :, h * 512 + b * 128 : h * 512 + (b + 1) * 128
                                        ],
                                        xe_sb[p][:],
                                        start=(p == 0),
                                        stop=(p == NP - 1),
                                        perf_mode=DR,
                                    )
                            # drains split across Act + DVE so PSUM frees fast
                            for b in range(4):
                                stg = p2st.tile([128, XW], f32, name="stg", tag="stg")
                                if b % 2 == 0:
                                    nc.scalar.copy(stg[:], segs[b][:, 0:XW])
                                else:
                                    nc.vector.tensor_copy(stg[:], segs[b][:, 0:XW])
                                kt = g * 8 + h * 4 + b
                                nc.sync.dma_start(
                                    partial_dram[kt * 128 : (kt + 1) * 128, :], stg[:]
                                )
                        # per-group ReduceScatter: overlaps later groups' matmuls on PE.
                        # rank i receives rows [g*1024 + i*128, +128) -> ccout[g*128:(g+1)*128]
                        if sim_single_core:
                            # TimelineSim/CoreSim stand-in: rank-0 shard copy.
                            nc.sync.dma_start(
                                ccout_dram[g * 128 : (g + 1) * 128, :],
                                partial_dram[g * 1024 : g * 1024 + 128, :],
                            )
                        else:
                            nc.gpsimd.collective_compute(
                                "ReduceScatter",
                                mybir.AluOpType.add,
                                replica_groups=[list(range(NCORES))],
                                ins=[partial_dram[g * 1024 : (g + 1) * 1024, :].opt()],
                                outs=[ccout_dram[g * 128 : (g + 1) * 128, :].opt()],
                            )
                        st = g
                        red = p3sb.tile([128, XW], f32, name="red", tag="red")
                        nc.sync.dma_start(red[:], ccout_dram[st * 128 : (st + 1) * 128, :])
                        dsum_t = p3sb.tile([128, C], f32, name="dsum_t", tag="dsum_t")
                        nc.sync.dma_start(dsum_t[:], dsum[st * 128 : (st + 1) * 128, :])
                        dnum_t = p3sb.tile([128, 1], f32, name="dnum_t", tag="dnum_t")
                        nc.sync.dma_start(dnum_t[:], dnum[st, :, :])
                        dsh_t = p3sb.tile([128, C], f32, name="dsh_t", tag="dsh_t")
                        nc.sync.dma_start(dsh_t[:], dsh[st * 128 : (st + 1) * 128, :])

                        cnt = red[:, C : C + 1]
                        maskb = p3sb.tile([128, 1], f32, name="maskb", tag="maskb")
                        nc.vector.tensor_scalar(
                            maskb[:], cnt, 0.0, None, mybir.AluOpType.is_gt
                        )
                        mask001 = p3sb.tile([128, 1], f32, name="mask001", tag="mask001")
                        nc.vector.tensor_scalar(
                            mask001[:], cnt, 0.0, 1.0 - MOM,
                            mybir.AluOpType.is_gt, mybir.AluOpType.mult,
                        )
                        tmp = p3sb.tile([128, C], f32, name="tmp", tag="tmp")
                        nc.vector.tensor_tensor(
                            tmp[:], red[:, 0:C], dsum_t[:], mybir.AluOpType.subtract
                        )
                        nc.vector.tensor_scalar(
                            tmp[:], tmp[:], mask001[:, 0:1], None, mybir.AluOpType.mult
                        )
                        nsum = p3sb.tile([128, C], f32, name="nsum", tag="nsum")
                        nc.vector.tensor_tensor(
                            nsum[:], tmp[:], dsum_t[:], mybir.AluOpType.add
                        )
                        n0 = p3sb.tile([128, 1], f32, name="n0", tag="n0")
                        nc.vector.tensor_tensor(
                            n0[:], cnt, dnum_t[:], mybir.AluOpType.subtract
                        )
                        nc.vector.tensor_tensor(
                            n0[:], n0[:], mask001[:], mybir.AluOpType.mult
                        )
                        nnum = p3sb.tile([128, 1], f32, name="nnum", tag="nnum")
                        nc.vector.tensor_tensor(
                            nnum[:], n0[:], dnum_t[:], mybir.AluOpType.add
                        )
                        rec = p3sb.tile([128, 1], f32, name="recq", tag="recq")
                        nc.vector.reciprocal(rec[:], nnum[:])
                        q = p3sb.tile([128, C], f32, name="q", tag="q")
                        nc.vector.tensor_scalar(
                            q[:], nsum[:], rec[:, 0:1], None, mybir.AluOpType.mult
                        )
                        nc.vector.tensor_tensor(
                            q[:], q[:], dsh_t[:], mybir.AluOpType.subtract
                        )
                        nc.vector.tensor_scalar(
                            q[:], q[:], maskb[:, 0:1], None, mybir.AluOpType.mult
                        )
                        outt = p3sb.tile([128, C], f32, name="outt", tag="outt")
                        nc.vector.tensor_tensor(
                            outt[:], q[:], dsh_t[:], mybir.AluOpType.add
                        )
                        nc.sync.dma_start(
                            out_shard[st * 128 : (st + 1) * 128, :], outt[:]
                        )

    nc.compile()
    return nc


def _shard_rows(i):
    """Global dictionary rows owned by core i: the i-th 128-block of each group."""
    return [(g * KSH + i * 128, g * KSH + i * 128 + 128) for g in range(KSH // 128)]


def shard_inputs(feature, dictionary, dictionary_sum, dictionary_num):
    in_maps = []
    for i in range(NCORES):
        rows = _shard_rows(i)
        dsum_i = np.concatenate([dictionary_sum[a:b] for a, b in rows], axis=0)
        dsh_i = np.concatenate([dictionary[a:b] for a, b in rows], axis=0)
        dnum_i = np.concatenate([dictionary_num[a:b] for a, b in rows], axis=0)
        in_maps.append(
            {
                "feat": np.ascontiguousarray(feature[i * NSH : (i + 1) * NSH]),
                "dic": dictionary,
                "dsum": np.ascontiguousarray(dsum_i),
                "dnum": np.ascontiguousarray(dnum_i).reshape(KSH // 128, 128, 1),
                "dsh": np.ascontiguousarray(dsh_i),
            }
        )
    return in_maps


def unshard_output(results):
    out = np.empty((K, C), np.float32)
    for i in range(NCORES):
        rows = _shard_rows(i)
        for g, (a, b) in enumerate(rows):
            out[a:b] = results[i]["out_shard"][g * 128 : (g + 1) * 128]
    return out


def kernel(feature, dictionary, dictionary_sum, dictionary_num):
    from concourse import bass_utils

    feature = np.ascontiguousarray(feature, dtype=np.float32)
    dictionary = np.ascontiguousarray(dictionary, dtype=np.float32)
    dictionary_sum = np.ascontiguousarray(dictionary_sum, dtype=np.float32)
    dictionary_num = np.ascontiguousarray(dictionary_num, dtype=np.float32)

    nc = _build()
    in_maps = shard_inputs(feature, dictionary, dictionary_sum, dictionary_num)
    res = bass_utils.run_bass_kernel_spmd(nc, in_maps, core_ids=list(range(NCORES)))
    return unshard_output(res.results).astype(np.float32)

